# revision 1
# baseline (speedup 1.0000x reference)
"""GPT2 attention (B=4, S=2048, D=768, H=12, no causal mask) on 8 trn2 cores.

Sharding: core c -> batch b = c//2, head-group g = c%2 (6 heads of 64).
Each core computes its 6 heads' attention + the matching row-block of the
output projection; host sums the two per-batch partials and adds b_proj.

On-chip layout (per core):
  xT   [768, 2048]  (6 tiles [128, S])  -- x transposed via PE transpose
  qT/kT [384, 2048] (3 tiles [128, S], head pair per tile, scaled by 1/8 (q))
  vaug [128, 6, 16, 65] -- v natural [sk, hd] per (head, sk-chunk) + ones col
  scoresT per (pair, sq-block, sk-chunk): psum [128 sk, 512 sq] (2 heads
  packed in the PE array via row tiling), exp'd by ACT into SBUF, then
  attnT accumulation matmul with vaug (M=65: row 64 = softmax denominator).
  Normalize: DVE reciprocal + gpsimd partition_broadcast + DVE multiply.
  proj: attnT is already the natural lhsT; out [S, 768] partial to DRAM.
"""

import json
from contextlib import ExitStack

import ml_dtypes
import numpy as np

import concourse.bass as bass
import concourse.mybir as mybir
import concourse.tile as tile
from concourse import library_config
from concourse.bass_utils import run_bass_kernel_spmd
from concourse.masks import make_identity

B, S, D = 4, 2048, 768
H, HD = 12, 64
HPC = 6            # heads per core
DKC = HPC * HD     # 384: per-core width of q/k/v
NPAIR = HPC // 2   # 3 head pairs
P = 128
F32 = mybir.dt.float32
BF16 = mybir.dt.bfloat16

NSQ = S // 512     # 4 sq blocks
NST = S // 128     # 16 s tiles
NDC = D // 128     # 6 d chunks


def _split_multi_waits(bir_bytes):
    """Walrus in this toolchain accepts only one sync-wait per instruction.

    Hoist extra waits onto same-engine NoOps inserted just before. Engines
    execute their stream in order and semaphores are monotonic, so this is
    semantically identical.
    """
    m = json.loads(bir_bytes)
    n_split = 0
    for fn in m["functions"]:
        for blk in fn["blocks"]:
            new = []
            for ins in blk["instructions"]:
                si = ins.get("sync_info")
                waits = (si or {}).get("on_wait") or []
                if len(waits) > 1:
                    n_split += 1
                    for j, w in enumerate(waits[:-1]):
                        new.append({
                            "debug": ins.get("debug", 0),
                            "engine": ins["engine"],
                            "ins": [], "outs": [],
                            "name": f"{ins['name']}w{j}",
                            "opcode": "NoOp",
                            "sync_info": {"on_update": [], "on_wait": [w]},
                        })
                    si["on_wait"] = [waits[-1]]
                new.append(ins)
            blk["instructions"] = new
    return json.dumps(m).encode()


def build_kernel():
    nc = bass.Bass("TRN2", target_bir_lowering=False, debug=False)
    x_d = nc.dram_tensor("x", [S, D], BF16, kind="ExternalInput").ap()
    wqkv_d = nc.dram_tensor("wqkv", [D, 3 * DKC], BF16, kind="ExternalInput").ap()
    bqkv_d = nc.dram_tensor("bqkv", [3 * DKC], F32, kind="ExternalInput").ap()
    wproj_d = nc.dram_tensor("wproj", [DKC, D], BF16, kind="ExternalInput").ap()
    out_d = nc.dram_tensor("out", [S, D], F32, kind="ExternalOutput").ap()

    with tile.TileContext(nc) as tc:
        with ExitStack() as ctx:
            _body(ctx, tc, x_d, wqkv_d, bqkv_d, wproj_d, out_d)
    orig_to_json = nc.to_json_bytes
    nc.to_json_bytes = lambda: _split_multi_waits(orig_to_json())
    return nc


def _body(ctx, tc, x_d, wqkv_d, bqkv_d, wproj_d, out_d):
    nc = tc.nc
    ADD = mybir.AluOpType.add
    MULT = mybir.AluOpType.mult
    EXP = mybir.ActivationFunctionType.Exp

    consts = ctx.enter_context(tc.tile_pool(name="consts", bufs=1))
    big = ctx.enter_context(tc.tile_pool(name="big", bufs=1))
    psA = ctx.enter_context(tc.tile_pool(name="psA", bufs=4, space="PSUM"))
    psB = ctx.enter_context(tc.tile_pool(name="psB", bufs=4, space="PSUM"))
    # phase-1/2-only pools, released after qkv to make room for attention
    phase1 = ExitStack()
    wpool = phase1.enter_context(tc.tile_pool(name="wpool", bufs=1))
    xTp = phase1.enter_context(tc.tile_pool(name="xTp", bufs=1))
    xnat = phase1.enter_context(tc.tile_pool(name="xnat", bufs=8))

    # --- constants / weights ---
    ident = consts.tile([P, P], BF16)
    make_identity(nc, ident)
    ones_row = consts.tile([1, P], BF16)
    nc.vector.memset(ones_row, 1.0)

    w_sb = []
    for c in range(NDC):
        t = wpool.tile([P, 3 * DKC], BF16, name=f"w_sb{c}")
        nc.sync.dma_start(out=t, in_=wqkv_d[c * P:(c + 1) * P, :])
        w_sb.append(t)
    wproj_sb = []
    for t3 in range(3):
        t = big.tile([P, D], BF16, name=f"wproj_sb{t3}")
        nc.sync.dma_start(out=t, in_=wproj_d[t3 * P:(t3 + 1) * P, :])
        wproj_sb.append(t)

    bq_sb = consts.tile([P, 3], F32)
    nc.sync.dma_start(out=bq_sb, in_=bqkv_d[0:DKC].rearrange("(t p) -> p t", p=P))
    bk_sb = consts.tile([P, 3], F32)
    nc.sync.dma_start(out=bk_sb, in_=bqkv_d[DKC:2 * DKC].rearrange("(t p) -> p t", p=P))
    bv_row = consts.tile([1, DKC], F32)
    nc.sync.dma_start(out=bv_row, in_=bqkv_d[2 * DKC:3 * DKC].rearrange("(o f) -> o f", o=1))
    bv16 = consts.tile([1, DKC], BF16)
    nc.vector.tensor_copy(out=bv16, in_=bv_row)

    # --- load x natural, transpose to xT ---
    xT = [xTp.tile([P, S], BF16, name=f"xT{c}") for c in range(NDC)]
    for sb in range(NSQ):
        xn = []
        for i in range(4):
            st = sb * 4 + i
            t = xnat.tile([P, D], BF16, name="xn", tag="xn")
            nc.sync.dma_start(out=t, in_=x_d[st * P:(st + 1) * P, :])
            xn.append(t)
        for c in range(NDC):
            pst = psA.tile([P, 512], BF16, name="pst", tag="psA")
            for i in range(4):
                nc.tensor.transpose(
                    pst[:, i * P:(i + 1) * P], xn[i][:, c * P:(c + 1) * P], ident)
            nc.vector.tensor_copy(out=xT[c][:, sb * 512:(sb + 1) * 512], in_=pst)

    # --- qkv projections ---
    qT = [big.tile([P, S], BF16, name=f"qT{t}") for t in range(3)]
    kT = [big.tile([P, S], BF16, name=f"kT{t}") for t in range(3)]
    for t3 in range(3):
        for sb in range(NSQ):
            for which in range(2):  # 0 = q, 1 = k
                ps = psB.tile([P, 512], F32, name="psqk", tag="psB")
                for c in range(NDC):
                    nc.tensor.matmul(
                        ps,
                        lhsT=(w_sb[c][:, which * DKC + t3 * P:which * DKC + (t3 + 1) * P]),
                        rhs=(xT[c][:, sb * 512:(sb + 1) * 512]),
                        start=(c == 0), stop=(c == NDC - 1))
                if which == 0:
                    nc.vector.tensor_scalar(
                        out=qT[t3][:, sb * 512:(sb + 1) * 512], in0=ps,
                        scalar1=bq_sb[:, t3:t3 + 1], scalar2=0.125, op0=ADD, op1=MULT)
                else:
                    nc.vector.tensor_scalar(
                        out=kT[t3][:, sb * 512:(sb + 1) * 512], in0=ps,
                        scalar1=bk_sb[:, t3:t3 + 1], scalar2=None, op0=ADD)

    vaug = big.tile([P, HPC, NST, P], BF16, name="vaug")
    nc.vector.memset(vaug[:, :, :, 64:128], 1.0)
    for st in range(NST):
        ps = psB.tile([P, DKC], F32, name="psv", tag="psB")
        for c in range(NDC):
            nc.tensor.matmul(
                ps,
                lhsT=(xT[c][:, st * P:(st + 1) * P]),
                rhs=(w_sb[c][:, 2 * DKC:3 * DKC]),
                start=(c == 0), stop=False)
        nc.tensor.matmul(ps, lhsT=ones_row, rhs=bv16, start=False, stop=True)
        nc.vector.tensor_copy(
            out=vaug[:, :, st, 0:64],
            in_=ps.rearrange("p (h e) -> p h e", h=HPC))

    # --- attention ---
    phase1.close()  # free xnat, xT, w_sb for the attention-phase pools
    att_pool = ctx.enter_context(tc.tile_pool(name="att_pool", bufs=1))
    expp = ctx.enter_context(tc.tile_pool(name="expp", bufs=4))
    smalls = ctx.enter_context(tc.tile_pool(name="smalls", bufs=2))
    outst = ctx.enter_context(tc.tile_pool(name="outst", bufs=3))
    attnT = [att_pool.tile([P, S], BF16, name=f"attnT{pr}") for pr in range(NPAIR)]

    def emit_av(pr, sb, ck, eA, eB, accA, accB):
        nc.tensor.matmul(
            accA, lhsT=(vaug[:, 2 * pr, ck, :]), rhs=(eA),
            start=(ck == 0), stop=(ck == NST - 1))
        nc.tensor.matmul(
            accB, lhsT=(vaug[:, 2 * pr + 1, ck, :]), rhs=(eB),
            start=(ck == 0), stop=(ck == NST - 1))

    for sb in range(NSQ):
        for pr in range(NPAIR):
            accA = psB.tile([P, 512], F32, name="accA", tag="psB")
            accB = psB.tile([P, 512], F32, name="accB", tag="psB")
            prev = None
            for ck in range(NST):
                sA = psA.tile([P, 512], F32, name="sA", tag="psA")
                sB = psA.tile([P, 512], F32, name="sB", tag="psA")
                nc.tensor.matmul(
                    sA, lhsT=(kT[pr][0:64, ck * P:(ck + 1) * P]),
                    rhs=(qT[pr][0:64, sb * 512:(sb + 1) * 512]),
                    start=True, stop=True, tile_position=(0, 0))
                nc.tensor.matmul(
                    sB, lhsT=(kT[pr][64:128, ck * P:(ck + 1) * P]),
                    rhs=(qT[pr][64:128, sb * 512:(sb + 1) * 512]),
                    start=True, stop=True, tile_position=(64, 0))
                if prev is not None:
                    emit_av(pr, sb, prev[0], prev[1], prev[2], accA, accB)
                eA = expp.tile([P, 512], BF16, name="eA", tag="expp")
                eB = expp.tile([P, 512], BF16, name="eB", tag="expp")
                nc.scalar.activation(out=eA, in_=sA, func=EXP)
                nc.scalar.activation(out=eB, in_=sB, func=EXP)
                prev = (ck, eA, eB)
            emit_av(pr, sb, prev[0], prev[1], prev[2], accA, accB)

            for hh, acc in ((0, accA), (1, accB)):
                rec = smalls.tile([64, 512], F32, name="rec", tag="rec")
                nc.vector.reciprocal(out=rec, in_=acc[64:128, :])
                nc.vector.tensor_tensor(
                    out=attnT[pr][64 * hh:64 * (hh + 1), sb * 512:(sb + 1) * 512],
                    in0=acc[0:64, :], in1=rec, op=MULT)

        # --- proj for this sq block ---
        for i in range(4):
            st = sb * 4 + i
            ostg = outst.tile([P, D], F32, name="ostg", tag="ostg")
            for half in range(2):
                pp = psB.tile([P, 384], F32, name="pp", tag="psB")
                for t3 in range(3):
                    nc.tensor.matmul(
                        pp,
                        lhsT=(attnT[t3][:, st * P:(st + 1) * P]),
                        rhs=(wproj_sb[t3][:, half * 384:(half + 1) * 384]),
                        start=(t3 == 0), stop=(t3 == 2))
                nc.vector.tensor_copy(out=ostg[:, half * 384:(half + 1) * 384], in_=pp)
            nc.sync.dma_start(out=out_d[st * P:(st + 1) * P, :], in_=ostg)


_NC_CACHE = None


def _get_nc():
    global _NC_CACHE
    if _NC_CACHE is None:
        _NC_CACHE = build_kernel()
    return _NC_CACHE


def make_in_maps(hidden_states, W_attn, b_attn, W_proj, b_proj):
    in_maps = []
    for c in range(8):
        b, g = c // 2, c % 2
        cols = slice(g * DKC, (g + 1) * DKC)
        wq = W_attn[:, 0 * D:1 * D][:, cols]
        wk = W_attn[:, 1 * D:2 * D][:, cols]
        wv = W_attn[:, 2 * D:3 * D][:, cols]
        bq = b_attn[0 * D:1 * D][cols]
        bk = b_attn[1 * D:2 * D][cols]
        bv = b_attn[2 * D:3 * D][cols]
        in_maps.append({
            "x": np.ascontiguousarray(hidden_states[b]).astype(ml_dtypes.bfloat16),
            "wqkv": np.ascontiguousarray(
                np.concatenate([wq, wk, wv], axis=1)).astype(ml_dtypes.bfloat16),
            "bqkv": np.ascontiguousarray(
                np.concatenate([bq, bk, bv]), dtype=np.float32),
            "wproj": np.ascontiguousarray(
                W_proj[g * DKC:(g + 1) * DKC, :]).astype(ml_dtypes.bfloat16),
        })
    return in_maps


def run(hidden_states, W_attn, b_attn, W_proj, b_proj, trace=False):
    nc = _get_nc()
    in_maps = make_in_maps(hidden_states, W_attn, b_attn, W_proj, b_proj)
    res = run_bass_kernel_spmd(nc, in_maps, core_ids=list(range(8)), trace=trace)
    out = np.empty((B, S, D), dtype=np.float32)
    for b in range(B):
        out[b] = res.results[2 * b]["out"] + res.results[2 * b + 1]["out"] + b_proj
    return out, res


def kernel(hidden_states, W_attn, b_attn, W_proj, b_proj):
    hidden_states = np.asarray(hidden_states, dtype=np.float32)
    W_attn = np.asarray(W_attn, dtype=np.float32)
    b_attn = np.asarray(b_attn, dtype=np.float32)
    W_proj = np.asarray(W_proj, dtype=np.float32)
    b_proj = np.asarray(b_proj, dtype=np.float32)
    out, _ = run(hidden_states, W_attn, b_attn, W_proj, b_proj, trace=False)
    return out



# revision 7
# speedup vs baseline: 1.1309x; 1.1309x over previous
"""GPT2 attention (B=4, S=2048, D=768, H=12, no causal mask) on 8 trn2 cores.

Sharding: core c -> batch b = c//2, head-group g = c%2 (6 heads of 64).
Each core computes its 6 heads' attention + the matching row-block of the
output projection; host sums the two per-batch partials and adds b_proj.

v2 pipeline (vs v1):
  - x arrives pre-transposed from host (xT [768, 2048] bf16): no PE
    transposes, no natural-x staging.
  - attention is head-pair-OUTER: for pr in 3: for sb in 4: ... so qk/v
    for pair p+1 and proj for finished rows interleave into the tensor
    stream while ScalarE (the true bottleneck: S*S*H/8 = 25.2M exps/core)
    stays saturated.
  - scores land as BF16 in [128, 2048] 2-bank PSUM regions (4 tiles of
    [128,512] per region, concurrent head-pair MMs split across banks);
    ONE activation per region: (2048+352)/1.2 ~ 2.0us per 4 tiles vs
    4 x 720ns unbatched.
  - softmax denominator rides the AV matmul (vaug ones column, M=65).
  - normalize uses reciprocal_approx_fast (~5x faster than reciprocal).
"""

import json
from contextlib import ExitStack

import ml_dtypes
import numpy as np

import concourse.bass as bass
import concourse.mybir as mybir
import concourse.tile as tile
from concourse.bass_utils import run_bass_kernel_spmd

B, S, D = 4, 2048, 768
H, HD = 12, 64
HPC = 6            # heads per core
DKC = HPC * HD     # 384: per-core width of q/k/v
NPAIR = HPC // 2   # 3 head pairs
P = 128
F32 = mybir.dt.float32
BF16 = mybir.dt.bfloat16

NSQ = S // 512     # 4 sq blocks
NST = S // 128     # 16 s tiles
NDC = D // 128     # 6 d chunks
NRG = NST // 2     # 8 score regions per (pair, sq-block), 2 ck-chunks each


def _split_multi_waits(bir_bytes):
    """Walrus in this toolchain accepts only one sync-wait per instruction.

    Hoist extra waits onto same-engine NoOps inserted just before. Engines
    execute their stream in order and semaphores are monotonic, so this is
    semantically identical.
    """
    m = json.loads(bir_bytes)
    for fn in m["functions"]:
        for blk in fn["blocks"]:
            new = []
            for ins in blk["instructions"]:
                si = ins.get("sync_info")
                waits = (si or {}).get("on_wait") or []
                if len(waits) > 1:
                    for j, w in enumerate(waits[:-1]):
                        new.append({
                            "debug": ins.get("debug", 0),
                            "engine": ins["engine"],
                            "ins": [], "outs": [],
                            "name": f"{ins['name']}w{j}",
                            "opcode": "NoOp",
                            "sync_info": {"on_update": [], "on_wait": [w]},
                        })
                    si["on_wait"] = [waits[-1]]
                new.append(ins)
            blk["instructions"] = new
    return json.dumps(m).encode()


def build_kernel():
    nc = bass.Bass("TRN2", target_bir_lowering=False, debug=False)
    xT_d = nc.dram_tensor("xT", [D, S], BF16, kind="ExternalInput").ap()
    wqkv_d = nc.dram_tensor("wqkv", [D, 3 * DKC], BF16, kind="ExternalInput").ap()
    bqkv_d = nc.dram_tensor("bqkv", [3 * DKC], F32, kind="ExternalInput").ap()
    wproj_d = nc.dram_tensor("wproj", [DKC, D], BF16, kind="ExternalInput").ap()
    out_d = nc.dram_tensor("out", [S, D], F32, kind="ExternalOutput").ap()

    with tile.TileContext(nc) as tc:
        with ExitStack() as ctx:
            _body(ctx, tc, xT_d, wqkv_d, bqkv_d, wproj_d, out_d)
    orig_to_json = nc.to_json_bytes
    nc.to_json_bytes = lambda: _split_multi_waits(orig_to_json())
    return nc


def _body(ctx, tc, xT_d, wqkv_d, bqkv_d, wproj_d, out_d):
    nc = tc.nc
    ADD = mybir.AluOpType.add
    MULT = mybir.AluOpType.mult
    EXP = mybir.ActivationFunctionType.Exp

    consts = ctx.enter_context(tc.tile_pool(name="consts", bufs=1))
    big = ctx.enter_context(tc.tile_pool(name="big", bufs=1))
    expp = ctx.enter_context(tc.tile_pool(name="expp", bufs=2))
    smalls = ctx.enter_context(tc.tile_pool(name="smalls", bufs=2))
    outst = ctx.enter_context(tc.tile_pool(name="outst", bufs=3))
    # PSUM: scoreP 2x2 banks + accA/accB 1 bank each + work 2x1 bank = 8
    scoreP = ctx.enter_context(tc.tile_pool(name="scoreP", bufs=2, space="PSUM"))
    accP = ctx.enter_context(tc.tile_pool(name="accP", bufs=1, space="PSUM"))
    workP = ctx.enter_context(tc.tile_pool(name="workP", bufs=2, space="PSUM"))

    # --- constants / weights / inputs ---
    ones_row = consts.tile([1, P], BF16)
    nc.vector.memset(ones_row, 1.0)

    xT = [big.tile([P, S], BF16, name=f"xT{c}") for c in range(NDC)]
    for c in range(NDC):
        nc.sync.dma_start(out=xT[c], in_=xT_d[c * P:(c + 1) * P, :])
    w_sb = []
    for c in range(NDC):
        t = big.tile([P, 3 * DKC], BF16, name=f"w_sb{c}")
        nc.sync.dma_start(out=t, in_=wqkv_d[c * P:(c + 1) * P, :])
        w_sb.append(t)
    wproj_sb = []
    for t3 in range(3):
        t = big.tile([P, D], BF16, name=f"wproj_sb{t3}")
        nc.sync.dma_start(out=t, in_=wproj_d[t3 * P:(t3 + 1) * P, :])
        wproj_sb.append(t)

    bq_sb = consts.tile([P, 3], F32)
    nc.sync.dma_start(out=bq_sb, in_=bqkv_d[0:DKC].rearrange("(t p) -> p t", p=P))
    bk_sb = consts.tile([P, 3], F32)
    nc.sync.dma_start(out=bk_sb, in_=bqkv_d[DKC:2 * DKC].rearrange("(t p) -> p t", p=P))
    bv_row = consts.tile([1, DKC], F32)
    nc.sync.dma_start(out=bv_row, in_=bqkv_d[2 * DKC:3 * DKC].rearrange("(o f) -> o f", o=1))
    bv16 = consts.tile([1, DKC], BF16)
    nc.vector.tensor_copy(out=bv16, in_=bv_row)

    qT = [big.tile([P, S], BF16, name=f"qT{t}") for t in range(3)]
    kT = [big.tile([P, S], BF16, name=f"kT{t}") for t in range(3)]
    vaug = big.tile([P, HPC, NST, P], BF16, name="vaug")
    nc.vector.memset(vaug[:, :, :, 64:128], 1.0)
    attnT = [big.tile([P, S], BF16, name=f"attnT{t}") for t in range(3)]

    # --- work emitters (each call emits one matmul group) ---
    def emit_qk(pr, sb, which):
        """qT/kT for pair pr, sq block sb. which: 0=q, 1=k."""
        ps = workP.tile([P, 512], F32, name="work", tag="work")
        for c in range(NDC):
            nc.tensor.matmul(
                ps,
                lhsT=(w_sb[c][:, which * DKC + pr * P:which * DKC + (pr + 1) * P]),
                rhs=(xT[c][:, sb * 512:(sb + 1) * 512]),
                start=(c == 0), stop=(c == NDC - 1))
        if which == 0:
            nc.vector.tensor_scalar(
                out=qT[pr][:, sb * 512:(sb + 1) * 512], in0=ps,
                scalar1=bq_sb[:, pr:pr + 1], scalar2=0.125, op0=ADD, op1=MULT)
        else:
            nc.vector.tensor_scalar(
                out=kT[pr][:, sb * 512:(sb + 1) * 512], in0=ps,
                scalar1=bk_sb[:, pr:pr + 1], scalar2=None, op0=ADD)

    def emit_v(pr, st):
        """v rows for s-tile st, pair pr (2 heads, cols pr*128:+128)."""
        ps = workP.tile([P, 512], F32, name="work", tag="work")
        pv = ps[:, 0:P]
        for c in range(NDC):
            nc.tensor.matmul(
                pv,
                lhsT=(xT[c][:, st * P:(st + 1) * P]),
                rhs=(w_sb[c][:, 2 * DKC + pr * P:2 * DKC + (pr + 1) * P]),
                start=(c == 0), stop=False)
        nc.tensor.matmul(
            pv, lhsT=ones_row, rhs=bv16[:, pr * P:(pr + 1) * P],
            start=False, stop=True)
        nc.vector.tensor_copy(
            out=vaug[:, 2 * pr:2 * pr + 2, st, 0:64],
            in_=pv.rearrange("p (h e) -> p h e", h=2))

    def emit_proj(sb, i, half):
        """proj output rows for s-tile st=sb*4+i, column half (384 wide)."""
        st = sb * 4 + i
        ps = workP.tile([P, 512], F32, name="work", tag="work")
        pp = ps[:, 0:384]
        for t3 in range(3):
            nc.tensor.matmul(
                pp,
                lhsT=(attnT[t3][:, st * P:(st + 1) * P]),
                rhs=(wproj_sb[t3][:, half * 384:(half + 1) * 384]),
                start=(t3 == 0), stop=(t3 == 2))
        ostg = _proj_stage[i]
        nc.vector.tensor_copy(out=ostg[:, half * 384:(half + 1) * 384], in_=pp)
        if half == 1:
            nc.sync.dma_start(out=out_d[st * P:(st + 1) * P, :], in_=ostg)

    _proj_stage = {}

    def proj_block(sb):
        """Emit one sb's proj as a list of thunks (interleaved later)."""
        thunks = []
        for i in range(4):
            def mk_stage(i=i):
                _proj_stage[i] = outst.tile([P, D], F32, name="ostg", tag="ostg")
            thunks.append(mk_stage)
            for half in range(2):
                thunks.append(lambda i=i, half=half: emit_proj(sb, i, half))
        return thunks

    def interleave_gen(pr):
        """Thunks to interleave into attention of pair pr: qk+v for pr+1,
        or (pr==2) nothing — proj is handled separately."""
        thunks = []
        if pr + 1 < NPAIR:
            for sb in range(NSQ):
                for which in range(2):
                    thunks.append(lambda s=sb, w=which: emit_qk(pr + 1, s, w))
            for st in range(NST):
                thunks.append(lambda s=st: emit_v(pr + 1, s))
        return thunks

    # --- prologue: qk + v for pair 0 ---
    for sb in range(NSQ):
        for which in range(2):
            emit_qk(0, sb, which)
    for st in range(NST):
        emit_v(0, st)

    # --- attention, pair-outer ---
    # region layout ([128, 1024] f32, 2 PSUM banks), one ck chunk:
    #   elems [0:512]    = h0   bank 0
    #   elems [512:1024] = h1   bank 1
    # concurrent row-tiled head pair lands in distinct banks.
    def score_mms(pr, sb, region, ck):
        """Scores for ck chunk of (pr, sb) into `region`."""
        nc.tensor.matmul(
            region[:, 0:512],
            lhsT=(kT[pr][0:64, ck * P:(ck + 1) * P]),
            rhs=(qT[pr][0:64, sb * 512:(sb + 1) * 512]),
            start=True, stop=True, tile_position=(0, 0))
        nc.tensor.matmul(
            region[:, 512:1024],
            lhsT=(kT[pr][64:128, ck * P:(ck + 1) * P]),
            rhs=(qT[pr][64:128, sb * 512:(sb + 1) * 512]),
            start=True, stop=True, tile_position=(64, 0))

    def av_mms(pr, ck, etile, accA, accB):
        """AV for ck chunk from exp tile `etile`."""
        nc.tensor.matmul(
            accA, lhsT=(vaug[:, 2 * pr, ck, :]),
            rhs=(etile[:, 0:512]),
            start=(ck == 0), stop=(ck == NST - 1))
        nc.tensor.matmul(
            accB, lhsT=(vaug[:, 2 * pr + 1, ck, :]),
            rhs=(etile[:, 512:1024]),
            start=(ck == 0), stop=(ck == NST - 1))

    for pr in range(NPAIR):
        side = interleave_gen(pr) if pr < 2 else []
        side_i = 0
        rg_count = 0
        proj_thunks = []
        for sb in range(NSQ):
            accA = accP.tile([P, 512], F32, name="accA", tag="accA")
            accB = accP.tile([P, 512], F32, name="accB", tag="accB")
            prev = None
            for j in range(NST):
                region = scoreP.tile([P, 1024], F32, name="score", tag="score")
                score_mms(pr, sb, region, j)
                # interleave qk/v for next pair (or proj of previous sb):
                # one side thunk every 3rd region keeps tensor ~balanced
                rg_count += 1
                if side_i < len(side) and rg_count % 3 == 0:
                    side[side_i]()
                    side_i += 1
                for _ in range(min(2, len(proj_thunks))):
                    proj_thunks.pop(0)()
                if prev is not None:
                    av_mms(pr, prev[0], prev[1], accA, accB)
                etile = expp.tile([P, 1024], BF16, name="etile", tag="etile")
                nc.scalar.activation(out=etile, in_=region, func=EXP)
                prev = (j, etile)
            av_mms(pr, prev[0], prev[1], accA, accB)

            # Stage accs to SBUF fast (frees the PSUM banks for the next
            # sq block), then the slow reciprocal runs off the critical path.
            stgs = []
            for hh, acc in ((0, accA), (1, accB)):
                stg = smalls.tile([P, 512], F32, name="stg", tag=f"stg{hh}")
                nc.vector.tensor_copy(out=stg, in_=acc)
                stgs.append(stg)
            for hh, stg in ((0, stgs[0]), (1, stgs[1])):
                rec = smalls.tile([64, 512], F32, name="rec", tag="rec")
                nc.vector.reciprocal(out=rec, in_=stg[64:128, :])
                nc.vector.tensor_tensor(
                    out=attnT[pr][64 * hh:64 * (hh + 1), sb * 512:(sb + 1) * 512],
                    in0=stg[0:64, :], in1=rec, op=MULT)
            if pr == 2:
                if sb < NSQ - 1:
                    proj_thunks = proj_block(sb)
                else:
                    for t in proj_block(sb):
                        t()
        # drain leftover side work before next pair needs it
        while side_i < len(side):
            side[side_i]()
            side_i += 1


_NC_CACHE = None


def _get_nc():
    global _NC_CACHE
    if _NC_CACHE is None:
        _NC_CACHE = build_kernel()
    return _NC_CACHE


def make_in_maps(hidden_states, W_attn, b_attn, W_proj, b_proj):
    in_maps = []
    for c in range(8):
        b, g = c // 2, c % 2
        cols = slice(g * DKC, (g + 1) * DKC)
        wq = W_attn[:, 0 * D:1 * D][:, cols]
        wk = W_attn[:, 1 * D:2 * D][:, cols]
        wv = W_attn[:, 2 * D:3 * D][:, cols]
        bq = b_attn[0 * D:1 * D][cols]
        bk = b_attn[1 * D:2 * D][cols]
        bv = b_attn[2 * D:3 * D][cols]
        in_maps.append({
            "xT": np.ascontiguousarray(hidden_states[b].T).astype(ml_dtypes.bfloat16),
            "wqkv": np.ascontiguousarray(
                np.concatenate([wq, wk, wv], axis=1)).astype(ml_dtypes.bfloat16),
            "bqkv": np.ascontiguousarray(
                np.concatenate([bq, bk, bv]), dtype=np.float32),
            "wproj": np.ascontiguousarray(
                W_proj[g * DKC:(g + 1) * DKC, :]).astype(ml_dtypes.bfloat16),
        })
    return in_maps


def run(hidden_states, W_attn, b_attn, W_proj, b_proj, trace=False):
    nc = _get_nc()
    in_maps = make_in_maps(hidden_states, W_attn, b_attn, W_proj, b_proj)
    res = run_bass_kernel_spmd(nc, in_maps, core_ids=list(range(8)), trace=trace)
    out = np.empty((B, S, D), dtype=np.float32)
    for b in range(B):
        out[b] = res.results[2 * b]["out"] + res.results[2 * b + 1]["out"] + b_proj
    return out, res


def kernel(hidden_states, W_attn, b_attn, W_proj, b_proj):
    hidden_states = np.asarray(hidden_states, dtype=np.float32)
    W_attn = np.asarray(W_attn, dtype=np.float32)
    b_attn = np.asarray(b_attn, dtype=np.float32)
    W_proj = np.asarray(W_proj, dtype=np.float32)
    b_proj = np.asarray(b_proj, dtype=np.float32)
    out, _ = run(hidden_states, W_attn, b_attn, W_proj, b_proj, trace=False)
    return out


# revision 12
# speedup vs baseline: 1.3421x; 1.1867x over previous
"""GPT2 attention (B=4, S=2048, D=768, H=12, no causal mask) on 8 trn2 cores.

Sharding: core c -> batch b = c//2, head-group g = c%2 (6 heads of 64).
Each core computes its 6 heads' attention + the matching row-block of the
output projection; host sums the two per-batch partials and adds b_proj.

v2 pipeline (vs v1):
  - x arrives pre-transposed from host (xT [768, 2048] bf16): no PE
    transposes, no natural-x staging.
  - attention is head-pair-OUTER: for pr in 3: for sb in 4: ... so qk/v
    for pair p+1 and proj for finished rows interleave into the tensor
    stream while ScalarE (the true bottleneck: S*S*H/8 = 25.2M exps/core)
    stays saturated.
  - scores land as BF16 in [128, 2048] 2-bank PSUM regions (4 tiles of
    [128,512] per region, concurrent head-pair MMs split across banks);
    ONE activation per region: (2048+352)/1.2 ~ 2.0us per 4 tiles vs
    4 x 720ns unbatched.
  - softmax denominator rides the AV matmul (vaug ones column, M=65).
  - normalize uses reciprocal_approx_fast (~5x faster than reciprocal).
"""

import json
from contextlib import ExitStack

import ml_dtypes
import numpy as np

import concourse.bass as bass
import concourse.mybir as mybir
import concourse.tile as tile
from concourse.bass_utils import run_bass_kernel_spmd

B, S, D = 4, 2048, 768
H, HD = 12, 64
HPC = 6            # heads per core
DKC = HPC * HD     # 384: per-core width of q/k/v
NPAIR = HPC // 2   # 3 head pairs
P = 128
F32 = mybir.dt.float32
BF16 = mybir.dt.bfloat16

NSQ = S // 512     # 4 sq blocks
NST = S // 128     # 16 s tiles
NDC = D // 128     # 6 d chunks
NRG = NST // 2     # 8 score regions per (pair, sq-block), 2 ck-chunks each


def _split_multi_waits(bir_bytes):
    """Walrus in this toolchain accepts only one sync-wait per instruction.

    Hoist extra waits onto same-engine NoOps inserted just before. Engines
    execute their stream in order and semaphores are monotonic, so this is
    semantically identical.
    """
    m = json.loads(bir_bytes)
    for fn in m["functions"]:
        for blk in fn["blocks"]:
            new = []
            for ins in blk["instructions"]:
                si = ins.get("sync_info")
                waits = (si or {}).get("on_wait") or []
                if len(waits) > 1:
                    for j, w in enumerate(waits[:-1]):
                        new.append({
                            "debug": ins.get("debug", 0),
                            "engine": ins["engine"],
                            "ins": [], "outs": [],
                            "name": f"{ins['name']}w{j}",
                            "opcode": "NoOp",
                            "sync_info": {"on_update": [], "on_wait": [w]},
                        })
                    si["on_wait"] = [waits[-1]]
                new.append(ins)
            blk["instructions"] = new
    return json.dumps(m).encode()


def build_kernel():
    nc = bass.Bass("TRN2", target_bir_lowering=False, debug=False)
    xT_d = nc.dram_tensor("xT", [D, S], BF16, kind="ExternalInput").ap()
    wqkv_d = nc.dram_tensor("wqkv", [D, 3 * DKC], BF16, kind="ExternalInput").ap()
    bqkv_d = nc.dram_tensor("bqkv", [3 * DKC], F32, kind="ExternalInput").ap()
    wproj_d = nc.dram_tensor("wproj", [DKC, D], BF16, kind="ExternalInput").ap()
    out_d = nc.dram_tensor("out", [S, D], F32, kind="ExternalOutput").ap()

    with tile.TileContext(nc) as tc:
        with ExitStack() as ctx:
            _body(ctx, tc, xT_d, wqkv_d, bqkv_d, wproj_d, out_d)
    orig_to_json = nc.to_json_bytes
    nc.to_json_bytes = lambda: _split_multi_waits(orig_to_json())
    return nc


def _body(ctx, tc, xT_d, wqkv_d, bqkv_d, wproj_d, out_d):
    nc = tc.nc
    ADD = mybir.AluOpType.add
    MULT = mybir.AluOpType.mult
    EXP = mybir.ActivationFunctionType.Exp

    consts = ctx.enter_context(tc.tile_pool(name="consts", bufs=1))
    big = ctx.enter_context(tc.tile_pool(name="big", bufs=1))
    expp = ctx.enter_context(tc.tile_pool(name="expp", bufs=2))
    smalls = ctx.enter_context(tc.tile_pool(name="smalls", bufs=2))
    outst = ctx.enter_context(tc.tile_pool(name="outst", bufs=3))
    # PSUM: scoreP 2x2 banks + accA/accB 1 bank each + work 2x1 bank = 8
    scoreP = ctx.enter_context(tc.tile_pool(name="scoreP", bufs=2, space="PSUM"))
    accP = ctx.enter_context(tc.tile_pool(name="accP", bufs=1, space="PSUM"))
    workP = ctx.enter_context(tc.tile_pool(name="workP", bufs=2, space="PSUM"))

    # --- constants / weights / inputs ---
    ones_row = consts.tile([1, P], BF16)
    nc.vector.memset(ones_row, 1.0)

    xT = [big.tile([P, S], BF16, name=f"xT{c}") for c in range(NDC)]
    for c in range(NDC):
        nc.sync.dma_start(out=xT[c], in_=xT_d[c * P:(c + 1) * P, :])
    w_sb = []
    for c in range(NDC):
        t = big.tile([P, 3 * DKC], BF16, name=f"w_sb{c}")
        nc.sync.dma_start(out=t, in_=wqkv_d[c * P:(c + 1) * P, :])
        w_sb.append(t)
    wproj_sb = []
    for t3 in range(3):
        t = big.tile([P, D], BF16, name=f"wproj_sb{t3}")
        nc.sync.dma_start(out=t, in_=wproj_d[t3 * P:(t3 + 1) * P, :])
        wproj_sb.append(t)

    bq_sb = consts.tile([P, 3], F32)
    nc.sync.dma_start(out=bq_sb, in_=bqkv_d[0:DKC].rearrange("(t p) -> p t", p=P))
    bk_sb = consts.tile([P, 3], F32)
    nc.sync.dma_start(out=bk_sb, in_=bqkv_d[DKC:2 * DKC].rearrange("(t p) -> p t", p=P))
    bv_row = consts.tile([1, DKC], F32)
    nc.sync.dma_start(out=bv_row, in_=bqkv_d[2 * DKC:3 * DKC].rearrange("(o f) -> o f", o=1))
    bv16 = consts.tile([1, DKC], BF16)
    nc.vector.tensor_copy(out=bv16, in_=bv_row)

    qT = [big.tile([P, S], BF16, name=f"qT{t}") for t in range(3)]
    kT = [big.tile([P, S], BF16, name=f"kT{t}") for t in range(3)]
    # per-(pair, s-tile) vaug tiles: interleaved v writes for pair p+1 must
    # not create tile-level false deps against pair p's AV reads
    vaug = [[big.tile([P, 2, P], BF16, name=f"vaug{p}_{s}") for s in range(NST)]
            for p in range(NPAIR)]
    for p in range(NPAIR):
        for s_ in range(NST):
            nc.vector.memset(vaug[p][s_][:, :, 64:128], 1.0)
    attnT = [big.tile([P, S], BF16, name=f"attnT{t}") for t in range(3)]

    # --- work emitters (each call emits one matmul group) ---
    def emit_qk(pr, sb, which):
        """qT/kT for pair pr, sq block sb. which: 0=q, 1=k."""
        ps = workP.tile([P, 512], F32, name="work", tag="work")
        for c in range(NDC):
            nc.tensor.matmul(
                ps,
                lhsT=(w_sb[c][:, which * DKC + pr * P:which * DKC + (pr + 1) * P]),
                rhs=(xT[c][:, sb * 512:(sb + 1) * 512]),
                start=(c == 0), stop=(c == NDC - 1))
        if which == 0:
            nc.vector.tensor_scalar(
                out=qT[pr][:, sb * 512:(sb + 1) * 512], in0=ps,
                scalar1=bq_sb[:, pr:pr + 1], scalar2=0.125, op0=ADD, op1=MULT)
        else:
            nc.vector.tensor_scalar(
                out=kT[pr][:, sb * 512:(sb + 1) * 512], in0=ps,
                scalar1=bk_sb[:, pr:pr + 1], scalar2=None, op0=ADD)

    def emit_v(pr, st):
        """v rows for s-tile st, pair pr (2 heads, cols pr*128:+128)."""
        ps = workP.tile([P, 512], F32, name="work", tag="work")
        pv = ps[:, 0:P]
        for c in range(NDC):
            nc.tensor.matmul(
                pv,
                lhsT=(xT[c][:, st * P:(st + 1) * P]),
                rhs=(w_sb[c][:, 2 * DKC + pr * P:2 * DKC + (pr + 1) * P]),
                start=(c == 0), stop=False)
        nc.tensor.matmul(
            pv, lhsT=ones_row, rhs=bv16[:, pr * P:(pr + 1) * P],
            start=False, stop=True)
        nc.vector.tensor_copy(
            out=vaug[pr][st][:, :, 0:64],
            in_=pv.rearrange("p (h e) -> p h e", h=2))

    def emit_proj(sb, i, half):
        """proj output rows for s-tile st=sb*4+i, column half (384 wide)."""
        st = sb * 4 + i
        ps = workP.tile([P, 512], F32, name="work", tag="work")
        pp = ps[:, 0:384]
        for t3 in range(3):
            nc.tensor.matmul(
                pp,
                lhsT=(attnT[t3][:, st * P:(st + 1) * P]),
                rhs=(wproj_sb[t3][:, half * 384:(half + 1) * 384]),
                start=(t3 == 0), stop=(t3 == 2))
        ostg = _proj_stage[i]
        nc.vector.tensor_copy(out=ostg[:, half * 384:(half + 1) * 384], in_=pp)
        if half == 1:
            nc.sync.dma_start(out=out_d[st * P:(st + 1) * P, :], in_=ostg)

    _proj_stage = {}

    def proj_block(sb):
        """Emit one sb's proj as a list of thunks (interleaved later)."""
        thunks = []
        for i in range(4):
            def mk_stage(i=i):
                _proj_stage[i] = outst.tile([P, D], F32, name="ostg", tag="ostg")
            thunks.append(mk_stage)
            for half in range(2):
                thunks.append(lambda i=i, half=half: emit_proj(sb, i, half))
        return thunks

    def interleave_gen(pr):
        """Thunks to interleave into attention of pair pr: qk+v for pr+1.
        For pr 0 the list is fronted by pair 0's own v tiles (one fires per
        region of sb 0, each just ahead of the AV that consumes it)."""
        thunks = []
        if pr == 0:
            for st in range(NST):
                thunks.append(lambda s=st: emit_v(0, s))
        if pr + 1 < NPAIR:
            for sb in range(NSQ):
                for which in range(2):
                    thunks.append(lambda s=sb, w=which: emit_qk(pr + 1, s, w))
            for st in range(NST):
                thunks.append(lambda s=st: emit_v(pr + 1, s))
        return thunks

    # --- prologue: qk for pair 0 only; its v interleaves into sb 0 ---
    for sb in range(NSQ):
        for which in range(2):
            emit_qk(0, sb, which)

    # --- attention, pair-outer ---
    # region layout ([128, 1024] f32, 2 PSUM banks), one ck chunk:
    #   elems [0:512]    = h0   bank 0
    #   elems [512:1024] = h1   bank 1
    # concurrent row-tiled head pair lands in distinct banks.
    def score_mms(pr, sb, region, ck):
        """Scores for ck chunk of (pr, sb) into `region`."""
        nc.tensor.matmul(
            region[:, 0:512],
            lhsT=(kT[pr][0:64, ck * P:(ck + 1) * P]),
            rhs=(qT[pr][0:64, sb * 512:(sb + 1) * 512]),
            start=True, stop=True, tile_position=(0, 0))
        nc.tensor.matmul(
            region[:, 512:1024],
            lhsT=(kT[pr][64:128, ck * P:(ck + 1) * P]),
            rhs=(qT[pr][64:128, sb * 512:(sb + 1) * 512]),
            start=True, stop=True, tile_position=(64, 0))

    def av_mms(pr, ck, etile, accA, accB):
        """AV for ck chunk from exp tile `etile`."""
        nc.tensor.matmul(
            accA, lhsT=(vaug[pr][ck][:, 0, :]),
            rhs=(etile[:, 0:512]),
            start=(ck == 0), stop=(ck == NST - 1))
        nc.tensor.matmul(
            accB, lhsT=(vaug[pr][ck][:, 1, :]),
            rhs=(etile[:, 512:1024]),
            start=(ck == 0), stop=(ck == NST - 1))

    for pr in range(NPAIR):
        side = interleave_gen(pr) if pr < 2 else []
        side_i = 0
        rg_count = 0
        proj_thunks = []
        for sb in range(NSQ):
            accA = accP.tile([P, 512], F32, name="accA", tag="accA")
            accB = accP.tile([P, 512], F32, name="accB", tag="accB")
            prev = None
            for j in range(NST):
                region = scoreP.tile([P, 1024], F32, name="score", tag="score")
                score_mms(pr, sb, region, j)
                # interleave qk/v for next pair (or proj of previous sb):
                # pair 0's own v fires every region of sb 0 (AV(ck) needs
                # vaug[0][ck] by region ck+1); the rest every other region.
                rg_count += 1
                if side_i < len(side):
                    if pr == 0 and side_i < NST:
                        side[side_i]()
                        side_i += 1
                    elif rg_count % 2 == 0:
                        side[side_i]()
                        side_i += 1
                for _ in range(min(2, len(proj_thunks))):
                    proj_thunks.pop(0)()
                if prev is not None:
                    av_mms(pr, prev[0], prev[1], accA, accB)
                etile = expp.tile([P, 1024], BF16, name="etile", tag="etile")
                nc.scalar.activation(out=etile, in_=region, func=EXP)
                prev = (j, etile)
            av_mms(pr, prev[0], prev[1], accA, accB)

            # Stage accs to SBUF fast (frees the PSUM banks for the next
            # sq block), then the slow reciprocal runs off the critical path.
            stgs = []
            for hh, acc in ((0, accA), (1, accB)):
                stg = smalls.tile([P, 512], F32, name="stg", tag=f"stg{hh}")
                nc.vector.tensor_copy(out=stg, in_=acc)
                stgs.append(stg)
            for hh, stg in ((0, stgs[0]), (1, stgs[1])):
                rec = smalls.tile([64, 512], F32, name="rec", tag="rec")
                nc.vector.reciprocal(out=rec, in_=stg[64:128, :])
                nc.vector.tensor_tensor(
                    out=attnT[pr][64 * hh:64 * (hh + 1), sb * 512:(sb + 1) * 512],
                    in0=stg[0:64, :], in1=rec, op=MULT)
            if pr == 2:
                if sb < NSQ - 1:
                    proj_thunks = proj_block(sb)
                else:
                    for t in proj_block(sb):
                        t()
        # drain leftover side work before next pair needs it
        while side_i < len(side):
            side[side_i]()
            side_i += 1


_NC_CACHE = None


def _get_nc():
    global _NC_CACHE
    if _NC_CACHE is None:
        _NC_CACHE = build_kernel()
    return _NC_CACHE


def make_in_maps(hidden_states, W_attn, b_attn, W_proj, b_proj):
    in_maps = []
    for c in range(8):
        b, g = c // 2, c % 2
        cols = slice(g * DKC, (g + 1) * DKC)
        wq = W_attn[:, 0 * D:1 * D][:, cols]
        wk = W_attn[:, 1 * D:2 * D][:, cols]
        wv = W_attn[:, 2 * D:3 * D][:, cols]
        bq = b_attn[0 * D:1 * D][cols]
        bk = b_attn[1 * D:2 * D][cols]
        bv = b_attn[2 * D:3 * D][cols]
        in_maps.append({
            "xT": np.ascontiguousarray(hidden_states[b].T).astype(ml_dtypes.bfloat16),
            "wqkv": np.ascontiguousarray(
                np.concatenate([wq, wk, wv], axis=1)).astype(ml_dtypes.bfloat16),
            "bqkv": np.ascontiguousarray(
                np.concatenate([bq, bk, bv]), dtype=np.float32),
            "wproj": np.ascontiguousarray(
                W_proj[g * DKC:(g + 1) * DKC, :]).astype(ml_dtypes.bfloat16),
        })
    return in_maps


def run(hidden_states, W_attn, b_attn, W_proj, b_proj, trace=False):
    nc = _get_nc()
    in_maps = make_in_maps(hidden_states, W_attn, b_attn, W_proj, b_proj)
    res = run_bass_kernel_spmd(nc, in_maps, core_ids=list(range(8)), trace=trace)
    out = np.empty((B, S, D), dtype=np.float32)
    for b in range(B):
        out[b] = res.results[2 * b]["out"] + res.results[2 * b + 1]["out"] + b_proj
    return out, res


def kernel(hidden_states, W_attn, b_attn, W_proj, b_proj):
    hidden_states = np.asarray(hidden_states, dtype=np.float32)
    W_attn = np.asarray(W_attn, dtype=np.float32)
    b_attn = np.asarray(b_attn, dtype=np.float32)
    W_proj = np.asarray(W_proj, dtype=np.float32)
    b_proj = np.asarray(b_proj, dtype=np.float32)
    out, _ = run(hidden_states, W_attn, b_attn, W_proj, b_proj, trace=False)
    return out


# revision 14
# speedup vs baseline: 1.4152x; 1.0545x over previous
"""GPT2 attention (B=4, S=2048, D=768, H=12, no causal mask) on 8 trn2 cores.

Sharding: core c -> batch b = c//2, head-group g = c%2 (6 heads of 64).
Each core computes its 6 heads' attention + the matching row-block of the
output projection; host sums the two per-batch partials and adds b_proj.

v4 pipeline (vs v1 baseline, 486us -> ~350us):
  - x arrives pre-transposed from host (xT [768, 2048] bf16): no PE
    transposes, no natural-x staging.
  - attention is head-pair-OUTER: qk/v for pair p+1 and proj for finished
    rows interleave into the tensor stream while ScalarE (the true
    bottleneck: S*S*H/8 = 25.2M exps/core) stays saturated.
  - scores land in [128, 1024] f32 2-bank PSUM regions (one ck chunk,
    concurrent row-tiled head pair split across the 2 banks); ONE
    activation per region (~1.1us per 2 tiles vs 2 x 720ns unbatched).
  - softmax denominator rides the AV matmul (vaug ones column, M=65);
    per-(pair, s-tile) vaug tiles avoid tile-level false deps from the
    interleaved v writes of the next pair.
  - normalize: accs staged to SBUF (frees PSUM + AV path immediately);
    the slow DVE reciprocal and the attnT-dependent proj MMs are
    deferred into the middle of the NEXT sq block so the score/ACT
    pipeline never stalls at block boundaries.
"""

import json
from contextlib import ExitStack

import ml_dtypes
import numpy as np

import concourse.bass as bass
import concourse.mybir as mybir
import concourse.tile as tile
from concourse.bass_utils import run_bass_kernel_spmd

B, S, D = 4, 2048, 768
H, HD = 12, 64
HPC = 6            # heads per core
DKC = HPC * HD     # 384: per-core width of q/k/v
NPAIR = HPC // 2   # 3 head pairs
P = 128
F32 = mybir.dt.float32
BF16 = mybir.dt.bfloat16

NSQ = S // 512     # 4 sq blocks
NST = S // 128     # 16 s tiles
NDC = D // 128     # 6 d chunks
NRG = NST // 2     # 8 score regions per (pair, sq-block), 2 ck-chunks each


def _split_multi_waits(bir_bytes):
    """Walrus in this toolchain accepts only one sync-wait per instruction.

    Hoist extra waits onto same-engine NoOps inserted just before. Engines
    execute their stream in order and semaphores are monotonic, so this is
    semantically identical.
    """
    m = json.loads(bir_bytes)
    for fn in m["functions"]:
        for blk in fn["blocks"]:
            new = []
            for ins in blk["instructions"]:
                si = ins.get("sync_info")
                waits = (si or {}).get("on_wait") or []
                if len(waits) > 1:
                    for j, w in enumerate(waits[:-1]):
                        new.append({
                            "debug": ins.get("debug", 0),
                            "engine": ins["engine"],
                            "ins": [], "outs": [],
                            "name": f"{ins['name']}w{j}",
                            "opcode": "NoOp",
                            "sync_info": {"on_update": [], "on_wait": [w]},
                        })
                    si["on_wait"] = [waits[-1]]
                new.append(ins)
            blk["instructions"] = new
    return json.dumps(m).encode()


def build_kernel():
    nc = bass.Bass("TRN2", target_bir_lowering=False, debug=False)
    xT_d = nc.dram_tensor("xT", [D, S], BF16, kind="ExternalInput").ap()
    wqkv_d = nc.dram_tensor("wqkv", [D, 3 * DKC], BF16, kind="ExternalInput").ap()
    bqkv_d = nc.dram_tensor("bqkv", [3 * DKC], F32, kind="ExternalInput").ap()
    wproj_d = nc.dram_tensor("wproj", [DKC, D], BF16, kind="ExternalInput").ap()
    out_d = nc.dram_tensor("out", [S, D], F32, kind="ExternalOutput").ap()

    with tile.TileContext(nc) as tc:
        with ExitStack() as ctx:
            _body(ctx, tc, xT_d, wqkv_d, bqkv_d, wproj_d, out_d)
    orig_to_json = nc.to_json_bytes
    nc.to_json_bytes = lambda: _split_multi_waits(orig_to_json())
    return nc


def _body(ctx, tc, xT_d, wqkv_d, bqkv_d, wproj_d, out_d):
    nc = tc.nc
    ADD = mybir.AluOpType.add
    MULT = mybir.AluOpType.mult
    EXP = mybir.ActivationFunctionType.Exp

    consts = ctx.enter_context(tc.tile_pool(name="consts", bufs=1))
    big = ctx.enter_context(tc.tile_pool(name="big", bufs=1))
    expp = ctx.enter_context(tc.tile_pool(name="expp", bufs=2))
    smalls = ctx.enter_context(tc.tile_pool(name="smalls", bufs=2))
    outst = ctx.enter_context(tc.tile_pool(name="outst", bufs=3))
    # PSUM: scoreP 2x2 banks + accA/accB 1 bank each + work 2x1 bank = 8
    scoreP = ctx.enter_context(tc.tile_pool(name="scoreP", bufs=2, space="PSUM"))
    accP = ctx.enter_context(tc.tile_pool(name="accP", bufs=1, space="PSUM"))
    workP = ctx.enter_context(tc.tile_pool(name="workP", bufs=2, space="PSUM"))

    # --- constants / weights / inputs ---
    ones_row = consts.tile([1, P], BF16)
    nc.vector.memset(ones_row, 1.0)

    xT = [big.tile([P, S], BF16, name=f"xT{c}") for c in range(NDC)]
    for c in range(NDC):
        nc.sync.dma_start(out=xT[c], in_=xT_d[c * P:(c + 1) * P, :])
    w_sb = []
    for c in range(NDC):
        t = big.tile([P, 3 * DKC], BF16, name=f"w_sb{c}")
        nc.sync.dma_start(out=t, in_=wqkv_d[c * P:(c + 1) * P, :])
        w_sb.append(t)
    wproj_sb = []
    for t3 in range(3):
        t = big.tile([P, D], BF16, name=f"wproj_sb{t3}")
        nc.sync.dma_start(out=t, in_=wproj_d[t3 * P:(t3 + 1) * P, :])
        wproj_sb.append(t)

    bq_sb = consts.tile([P, 3], F32)
    nc.sync.dma_start(out=bq_sb, in_=bqkv_d[0:DKC].rearrange("(t p) -> p t", p=P))
    bk_sb = consts.tile([P, 3], F32)
    nc.sync.dma_start(out=bk_sb, in_=bqkv_d[DKC:2 * DKC].rearrange("(t p) -> p t", p=P))
    bv_row = consts.tile([1, DKC], F32)
    nc.sync.dma_start(out=bv_row, in_=bqkv_d[2 * DKC:3 * DKC].rearrange("(o f) -> o f", o=1))
    bv16 = consts.tile([1, DKC], BF16)
    nc.vector.tensor_copy(out=bv16, in_=bv_row)

    qT = [big.tile([P, S], BF16, name=f"qT{t}") for t in range(3)]
    kT = [big.tile([P, S], BF16, name=f"kT{t}") for t in range(3)]
    # per-(pair, s-tile) vaug tiles: interleaved v writes for pair p+1 must
    # not create tile-level false deps against pair p's AV reads
    vaug = [[big.tile([P, 2, P], BF16, name=f"vaug{p}_{s}") for s in range(NST)]
            for p in range(NPAIR)]
    for p in range(NPAIR):
        for s_ in range(NST):
            nc.vector.memset(vaug[p][s_][:, :, 64:128], 1.0)
    attnT = [big.tile([P, S], BF16, name=f"attnT{t}") for t in range(3)]

    # --- work emitters (each call emits one matmul group) ---
    def emit_qk(pr, sb, which):
        """qT/kT for pair pr, sq block sb. which: 0=q, 1=k."""
        ps = workP.tile([P, 512], F32, name="work", tag="work")
        for c in range(NDC):
            nc.tensor.matmul(
                ps,
                lhsT=(w_sb[c][:, which * DKC + pr * P:which * DKC + (pr + 1) * P]),
                rhs=(xT[c][:, sb * 512:(sb + 1) * 512]),
                start=(c == 0), stop=(c == NDC - 1))
        if which == 0:
            nc.vector.tensor_scalar(
                out=qT[pr][:, sb * 512:(sb + 1) * 512], in0=ps,
                scalar1=bq_sb[:, pr:pr + 1], scalar2=0.125, op0=ADD, op1=MULT)
        else:
            nc.vector.tensor_scalar(
                out=kT[pr][:, sb * 512:(sb + 1) * 512], in0=ps,
                scalar1=bk_sb[:, pr:pr + 1], scalar2=None, op0=ADD)

    def emit_v(pr, st):
        """v rows for s-tile st, pair pr (2 heads, cols pr*128:+128)."""
        ps = workP.tile([P, 512], F32, name="work", tag="work")
        pv = ps[:, 0:P]
        for c in range(NDC):
            nc.tensor.matmul(
                pv,
                lhsT=(xT[c][:, st * P:(st + 1) * P]),
                rhs=(w_sb[c][:, 2 * DKC + pr * P:2 * DKC + (pr + 1) * P]),
                start=(c == 0), stop=False)
        nc.tensor.matmul(
            pv, lhsT=ones_row, rhs=bv16[:, pr * P:(pr + 1) * P],
            start=False, stop=True)
        nc.vector.tensor_copy(
            out=vaug[pr][st][:, :, 0:64],
            in_=pv.rearrange("p (h e) -> p h e", h=2))

    def emit_proj(sb, i, half):
        """proj output rows for s-tile st=sb*4+i, column half (384 wide)."""
        st = sb * 4 + i
        ps = workP.tile([P, 512], F32, name="work", tag="work")
        pp = ps[:, 0:384]
        for t3 in range(3):
            nc.tensor.matmul(
                pp,
                lhsT=(attnT[t3][:, st * P:(st + 1) * P]),
                rhs=(wproj_sb[t3][:, half * 384:(half + 1) * 384]),
                start=(t3 == 0), stop=(t3 == 2))
        ostg = _proj_stage[i]
        nc.vector.tensor_copy(out=ostg[:, half * 384:(half + 1) * 384], in_=pp)
        if half == 1:
            nc.sync.dma_start(out=out_d[st * P:(st + 1) * P, :], in_=ostg)

    _proj_stage = {}

    def proj_block(sb):
        """Emit one sb's proj as a list of thunks (interleaved later)."""
        thunks = []
        for i in range(4):
            def mk_stage(i=i):
                _proj_stage[i] = outst.tile([P, D], F32, name="ostg", tag="ostg")
            thunks.append(mk_stage)
            for half in range(2):
                thunks.append(lambda i=i, half=half: emit_proj(sb, i, half))
        return thunks

    def interleave_gen(pr):
        """Thunks to interleave into attention of pair pr: qk+v for pr+1.
        For pr 0 the list is fronted by pair 0's own v tiles (one fires per
        region of sb 0, each just ahead of the AV that consumes it)."""
        thunks = []
        if pr == 0:
            for st in range(NST):
                thunks.append(lambda s=st: emit_v(0, s))
        if pr + 1 < NPAIR:
            for sb in range(NSQ):
                for which in range(2):
                    thunks.append(lambda s=sb, w=which: emit_qk(pr + 1, s, w))
            for st in range(NST):
                thunks.append(lambda s=st: emit_v(pr + 1, s))
        return thunks

    # --- prologue: qk for pair 0 only; its v interleaves into sb 0 ---
    for sb in range(NSQ):
        for which in range(2):
            emit_qk(0, sb, which)

    # --- attention, pair-outer ---
    # region layout ([128, 1024] f32, 2 PSUM banks), one ck chunk:
    #   elems [0:512]    = h0   bank 0
    #   elems [512:1024] = h1   bank 1
    # concurrent row-tiled head pair lands in distinct banks.
    def score_mms(pr, sb, region, ck):
        """Scores for ck chunk of (pr, sb) into `region`."""
        nc.tensor.matmul(
            region[:, 0:512],
            lhsT=(kT[pr][0:64, ck * P:(ck + 1) * P]),
            rhs=(qT[pr][0:64, sb * 512:(sb + 1) * 512]),
            start=True, stop=True, tile_position=(0, 0))
        nc.tensor.matmul(
            region[:, 512:1024],
            lhsT=(kT[pr][64:128, ck * P:(ck + 1) * P]),
            rhs=(qT[pr][64:128, sb * 512:(sb + 1) * 512]),
            start=True, stop=True, tile_position=(64, 0))

    def av_mms(pr, ck, etile, accA, accB):
        """AV for ck chunk from exp tile `etile`."""
        nc.tensor.matmul(
            accA, lhsT=(vaug[pr][ck][:, 0, :]),
            rhs=(etile[:, 0:512]),
            start=(ck == 0), stop=(ck == NST - 1))
        nc.tensor.matmul(
            accB, lhsT=(vaug[pr][ck][:, 1, :]),
            rhs=(etile[:, 512:1024]),
            start=(ck == 0), stop=(ck == NST - 1))

    def emit_norm(pr, sb, hh, stg):
        """attnT rows for head hh of (pr, sb) from the staged acc copy."""
        rec = smalls.tile([64, 512], F32, name="rec", tag="rec")
        nc.vector.reciprocal(out=rec, in_=stg[64:128, :])
        nc.vector.tensor_tensor(
            out=attnT[pr][64 * hh:64 * (hh + 1), sb * 512:(sb + 1) * 512],
            in0=stg[0:64, :], in1=rec, op=MULT)

    norm_thunks = []
    proj_thunks = []
    for pr in range(NPAIR):
        side = interleave_gen(pr) if pr < 2 else []
        side_i = 0
        rg_count = 0
        for sb in range(NSQ):
            accA = accP.tile([P, 512], F32, name="accA", tag="accA")
            accB = accP.tile([P, 512], F32, name="accB", tag="accB")
            prev = None
            for j in range(NST):
                region = scoreP.tile([P, 1024], F32, name="score", tag="score")
                score_mms(pr, sb, region, j)
                # interleave qk/v for next pair: pair 0's own v fires every
                # region of sb 0 (AV(ck) needs vaug[0][ck] by region ck+1);
                # the rest every other region.
                rg_count += 1
                if side_i < len(side):
                    if pr == 0 and side_i < NST:
                        side[side_i]()
                        side_i += 1
                    elif rg_count % 2 == 0:
                        side[side_i]()
                        side_i += 1
                # deferred normalize (regions 2-5) and proj (regions 6+) of
                # the PREVIOUS sq block: keeps the slow reciprocal chain and
                # the attnT-dependent proj MMs away from the boundary, where
                # they stall the score/ACT pipeline.
                if 2 <= j <= 5:
                    if norm_thunks:
                        norm_thunks.pop(0)()
                elif j >= 6:
                    for _ in range(min(2, len(proj_thunks))):
                        proj_thunks.pop(0)()
                if prev is not None:
                    av_mms(pr, prev[0], prev[1], accA, accB)
                etile = expp.tile([P, 1024], BF16, name="etile", tag="etile")
                nc.scalar.activation(out=etile, in_=region, func=EXP)
                prev = (j, etile)
            av_mms(pr, prev[0], prev[1], accA, accB)

            # Stage accs to SBUF fast (frees the PSUM banks for the next sq
            # block); the reciprocal+mult run deferred, off the critical path.
            for hh, acc in ((0, accA), (1, accB)):
                stg = smalls.tile([P, 512], F32, name="stg", tag=f"stg{hh}")
                nc.vector.tensor_copy(out=stg, in_=acc)
                norm_thunks.append(
                    lambda pr=pr, sb=sb, hh=hh, stg=stg: emit_norm(pr, sb, hh, stg))
            if pr == 2 and sb < NSQ - 1:
                proj_thunks.extend(proj_block(sb))
        # drain leftover side work before next pair needs it
        while side_i < len(side):
            side[side_i]()
            side_i += 1
    # final drain: last normalizes, then the last sq block's proj
    while norm_thunks:
        norm_thunks.pop(0)()
    for t in proj_block(NSQ - 1):
        t()
    while proj_thunks:
        proj_thunks.pop(0)()


_NC_CACHE = None


def _get_nc():
    global _NC_CACHE
    if _NC_CACHE is None:
        _NC_CACHE = build_kernel()
    return _NC_CACHE


def make_in_maps(hidden_states, W_attn, b_attn, W_proj, b_proj):
    in_maps = []
    for c in range(8):
        b, g = c // 2, c % 2
        cols = slice(g * DKC, (g + 1) * DKC)
        wq = W_attn[:, 0 * D:1 * D][:, cols]
        wk = W_attn[:, 1 * D:2 * D][:, cols]
        wv = W_attn[:, 2 * D:3 * D][:, cols]
        bq = b_attn[0 * D:1 * D][cols]
        bk = b_attn[1 * D:2 * D][cols]
        bv = b_attn[2 * D:3 * D][cols]
        in_maps.append({
            "xT": np.ascontiguousarray(hidden_states[b].T).astype(ml_dtypes.bfloat16),
            "wqkv": np.ascontiguousarray(
                np.concatenate([wq, wk, wv], axis=1)).astype(ml_dtypes.bfloat16),
            "bqkv": np.ascontiguousarray(
                np.concatenate([bq, bk, bv]), dtype=np.float32),
            "wproj": np.ascontiguousarray(
                W_proj[g * DKC:(g + 1) * DKC, :]).astype(ml_dtypes.bfloat16),
        })
    return in_maps


def run(hidden_states, W_attn, b_attn, W_proj, b_proj, trace=False):
    nc = _get_nc()
    in_maps = make_in_maps(hidden_states, W_attn, b_attn, W_proj, b_proj)
    res = run_bass_kernel_spmd(nc, in_maps, core_ids=list(range(8)), trace=trace)
    out = np.empty((B, S, D), dtype=np.float32)
    for b in range(B):
        out[b] = res.results[2 * b]["out"] + res.results[2 * b + 1]["out"] + b_proj
    return out, res


def kernel(hidden_states, W_attn, b_attn, W_proj, b_proj):
    hidden_states = np.asarray(hidden_states, dtype=np.float32)
    W_attn = np.asarray(W_attn, dtype=np.float32)
    b_attn = np.asarray(b_attn, dtype=np.float32)
    W_proj = np.asarray(W_proj, dtype=np.float32)
    b_proj = np.asarray(b_proj, dtype=np.float32)
    out, _ = run(hidden_states, W_attn, b_attn, W_proj, b_proj, trace=False)
    return out


# revision 16
# speedup vs baseline: 1.4216x; 1.0045x over previous
"""GPT2 attention (B=4, S=2048, D=768, H=12, no causal mask) on 8 trn2 cores.

Sharding: core c -> batch b = c//2, head-group g = c%2 (6 heads of 64).
Each core computes its 6 heads' attention + the matching row-block of the
output projection; host sums the two per-batch partials and adds b_proj.

v4 pipeline (vs v1 baseline, 486us -> ~350us):
  - x arrives pre-transposed from host (xT [768, 2048] bf16): no PE
    transposes, no natural-x staging.
  - attention is head-pair-OUTER: qk/v for pair p+1 and proj for finished
    rows interleave into the tensor stream while ScalarE (the true
    bottleneck: S*S*H/8 = 25.2M exps/core) stays saturated.
  - scores land in [128, 1024] f32 2-bank PSUM regions (one ck chunk,
    concurrent row-tiled head pair split across the 2 banks); ONE
    activation per region (~1.1us per 2 tiles vs 2 x 720ns unbatched).
  - softmax denominator rides the AV matmul (vaug ones column, M=65);
    per-(pair, s-tile) vaug tiles avoid tile-level false deps from the
    interleaved v writes of the next pair.
  - normalize: accs staged to SBUF (frees PSUM + AV path immediately);
    the slow DVE reciprocal and the attnT-dependent proj MMs are
    deferred into the middle of the NEXT sq block so the score/ACT
    pipeline never stalls at block boundaries.
"""

import json
from contextlib import ExitStack

import ml_dtypes
import numpy as np

import concourse.bass as bass
import concourse.mybir as mybir
import concourse.tile as tile
from concourse.bass_utils import run_bass_kernel_spmd

B, S, D = 4, 2048, 768
H, HD = 12, 64
HPC = 6            # heads per core
DKC = HPC * HD     # 384: per-core width of q/k/v
NPAIR = HPC // 2   # 3 head pairs
P = 128
F32 = mybir.dt.float32
BF16 = mybir.dt.bfloat16

NSQ = S // 512     # 4 sq blocks
NST = S // 128     # 16 s tiles
NDC = D // 128     # 6 d chunks
NRG = NST // 2     # 8 score regions per (pair, sq-block), 2 ck-chunks each


def _split_multi_waits(bir_bytes):
    """Walrus in this toolchain accepts only one sync-wait per instruction.

    Hoist extra waits onto same-engine NoOps inserted just before. Engines
    execute their stream in order and semaphores are monotonic, so this is
    semantically identical.
    """
    m = json.loads(bir_bytes)
    for fn in m["functions"]:
        for blk in fn["blocks"]:
            new = []
            for ins in blk["instructions"]:
                si = ins.get("sync_info")
                waits = (si or {}).get("on_wait") or []
                if len(waits) > 1:
                    for j, w in enumerate(waits[:-1]):
                        new.append({
                            "debug": ins.get("debug", 0),
                            "engine": ins["engine"],
                            "ins": [], "outs": [],
                            "name": f"{ins['name']}w{j}",
                            "opcode": "NoOp",
                            "sync_info": {"on_update": [], "on_wait": [w]},
                        })
                    si["on_wait"] = [waits[-1]]
                new.append(ins)
            blk["instructions"] = new
    return json.dumps(m).encode()


def build_kernel():
    nc = bass.Bass("TRN2", target_bir_lowering=False, debug=False)
    xT_d = nc.dram_tensor("xT", [D, S], BF16, kind="ExternalInput").ap()
    wqkv_d = nc.dram_tensor("wqkv", [D, 3 * DKC], BF16, kind="ExternalInput").ap()
    bqkv_d = nc.dram_tensor("bqkv", [3 * DKC], F32, kind="ExternalInput").ap()
    wproj_d = nc.dram_tensor("wproj", [DKC, D], BF16, kind="ExternalInput").ap()
    out_d = nc.dram_tensor("out", [S, D], F32, kind="ExternalOutput").ap()

    with tile.TileContext(nc) as tc:
        with ExitStack() as ctx:
            _body(ctx, tc, xT_d, wqkv_d, bqkv_d, wproj_d, out_d)
    orig_to_json = nc.to_json_bytes
    nc.to_json_bytes = lambda: _split_multi_waits(orig_to_json())
    return nc


def _body(ctx, tc, xT_d, wqkv_d, bqkv_d, wproj_d, out_d):
    nc = tc.nc
    ADD = mybir.AluOpType.add
    MULT = mybir.AluOpType.mult
    EXP = mybir.ActivationFunctionType.Exp

    consts = ctx.enter_context(tc.tile_pool(name="consts", bufs=1))
    big = ctx.enter_context(tc.tile_pool(name="big", bufs=1))
    expp = ctx.enter_context(tc.tile_pool(name="expp", bufs=2))
    smalls = ctx.enter_context(tc.tile_pool(name="smalls", bufs=2))
    outst = ctx.enter_context(tc.tile_pool(name="outst", bufs=3))
    # PSUM: scoreP 2x2 banks + accA/accB 1 bank each + work 2x1 bank = 8
    scoreP = ctx.enter_context(tc.tile_pool(name="scoreP", bufs=2, space="PSUM"))
    accP = ctx.enter_context(tc.tile_pool(name="accP", bufs=1, space="PSUM"))
    workP = ctx.enter_context(tc.tile_pool(name="workP", bufs=2, space="PSUM"))

    # --- constants / weights / inputs ---
    ones_row = consts.tile([1, P], BF16)
    nc.vector.memset(ones_row, 1.0)

    xT = [big.tile([P, S], BF16, name=f"xT{c}") for c in range(NDC)]
    for c in range(NDC):
        nc.sync.dma_start(out=xT[c], in_=xT_d[c * P:(c + 1) * P, :])
    w_sb = []
    for c in range(NDC):
        t = big.tile([P, 3 * DKC], BF16, name=f"w_sb{c}")
        nc.sync.dma_start(out=t, in_=wqkv_d[c * P:(c + 1) * P, :])
        w_sb.append(t)
    wproj_sb = []
    for t3 in range(3):
        t = big.tile([P, D], BF16, name=f"wproj_sb{t3}")
        nc.sync.dma_start(out=t, in_=wproj_d[t3 * P:(t3 + 1) * P, :])
        wproj_sb.append(t)

    bq_sb = consts.tile([P, 3], F32)
    nc.sync.dma_start(out=bq_sb, in_=bqkv_d[0:DKC].rearrange("(t p) -> p t", p=P))
    bk_sb = consts.tile([P, 3], F32)
    nc.sync.dma_start(out=bk_sb, in_=bqkv_d[DKC:2 * DKC].rearrange("(t p) -> p t", p=P))
    bv_row = consts.tile([1, DKC], F32)
    nc.sync.dma_start(out=bv_row, in_=bqkv_d[2 * DKC:3 * DKC].rearrange("(o f) -> o f", o=1))
    bv16 = consts.tile([1, DKC], BF16)
    nc.vector.tensor_copy(out=bv16, in_=bv_row)

    qT = [big.tile([P, S], BF16, name=f"qT{t}") for t in range(3)]
    kT = [big.tile([P, S], BF16, name=f"kT{t}") for t in range(3)]
    # per-(pair, s-tile) vaug tiles: interleaved v writes for pair p+1 must
    # not create tile-level false deps against pair p's AV reads
    vaug = [[big.tile([P, 2, P], BF16, name=f"vaug{p}_{s}") for s in range(NST)]
            for p in range(NPAIR)]
    for p in range(NPAIR):
        for s_ in range(NST):
            nc.vector.memset(vaug[p][s_][:, :, 64:128], 1.0)
    attnT = [big.tile([P, S], BF16, name=f"attnT{t}") for t in range(3)]

    # --- work emitters (each call emits one matmul group) ---
    def emit_qk(pr, sb, which):
        """qT/kT for pair pr, sq block sb. which: 0=q, 1=k."""
        ps = workP.tile([P, 512], F32, name="work", tag="work")
        for c in range(NDC):
            nc.tensor.matmul(
                ps,
                lhsT=(w_sb[c][:, which * DKC + pr * P:which * DKC + (pr + 1) * P]),
                rhs=(xT[c][:, sb * 512:(sb + 1) * 512]),
                start=(c == 0), stop=(c == NDC - 1))
        if which == 0:
            nc.vector.tensor_scalar(
                out=qT[pr][:, sb * 512:(sb + 1) * 512], in0=ps,
                scalar1=bq_sb[:, pr:pr + 1], scalar2=0.125, op0=ADD, op1=MULT)
        else:
            nc.vector.tensor_scalar(
                out=kT[pr][:, sb * 512:(sb + 1) * 512], in0=ps,
                scalar1=bk_sb[:, pr:pr + 1], scalar2=None, op0=ADD)

    def emit_v(pr, st):
        """v rows for s-tile st, pair pr (2 heads, cols pr*128:+128)."""
        ps = workP.tile([P, 512], F32, name="work", tag="work")
        pv = ps[:, 0:P]
        for c in range(NDC):
            nc.tensor.matmul(
                pv,
                lhsT=(xT[c][:, st * P:(st + 1) * P]),
                rhs=(w_sb[c][:, 2 * DKC + pr * P:2 * DKC + (pr + 1) * P]),
                start=(c == 0), stop=False)
        nc.tensor.matmul(
            pv, lhsT=ones_row, rhs=bv16[:, pr * P:(pr + 1) * P],
            start=False, stop=True)
        nc.vector.tensor_copy(
            out=vaug[pr][st][:, :, 0:64],
            in_=pv.rearrange("p (h e) -> p h e", h=2))

    def emit_proj(sb, i, half):
        """proj output rows for s-tile st=sb*4+i, column half (384 wide)."""
        st = sb * 4 + i
        ps = workP.tile([P, 512], F32, name="work", tag="work")
        pp = ps[:, 0:384]
        for t3 in range(3):
            nc.tensor.matmul(
                pp,
                lhsT=(attnT[t3][:, st * P:(st + 1) * P]),
                rhs=(wproj_sb[t3][:, half * 384:(half + 1) * 384]),
                start=(t3 == 0), stop=(t3 == 2))
        ostg = _proj_stage[i]
        nc.vector.tensor_copy(out=ostg[:, half * 384:(half + 1) * 384], in_=pp)
        if half == 1:
            nc.sync.dma_start(out=out_d[st * P:(st + 1) * P, :], in_=ostg)

    _proj_stage = {}

    def proj_block(sb):
        """Emit one sb's proj as a list of thunks (interleaved later)."""
        thunks = []
        for i in range(4):
            def mk_stage(i=i):
                _proj_stage[i] = outst.tile([P, D], F32, name="ostg", tag="ostg")
            thunks.append(mk_stage)
            for half in range(2):
                thunks.append(lambda i=i, half=half: emit_proj(sb, i, half))
        return thunks

    def interleave_gen(pr):
        """Thunks to interleave into attention of pair pr: qk+v for pr+1.
        For pr 0 the list is fronted by pair 0's own v tiles (one fires per
        region of sb 0, each just ahead of the AV that consumes it)."""
        thunks = []
        if pr == 0:
            for st in range(NST):
                thunks.append(lambda s=st: emit_v(0, s))
        if pr + 1 < NPAIR:
            for sb in range(NSQ):
                for which in range(2):
                    thunks.append(lambda s=sb, w=which: emit_qk(pr + 1, s, w))
            for st in range(NST):
                thunks.append(lambda s=st: emit_v(pr + 1, s))
        return thunks

    # --- prologue: qk for pair 0 only; its v interleaves into sb 0.
    # Order: q(sb0), then ALL of k (sb 0's scores need the full kT stripe
    # under coarse dep tracking), then the remaining q blocks. ---
    emit_qk(0, 0, 0)
    for sb in range(NSQ):
        emit_qk(0, sb, 1)
    for sb in range(1, NSQ):
        emit_qk(0, sb, 0)

    # --- attention, pair-outer ---
    # region layout ([128, 1024] f32, 2 PSUM banks), one ck chunk:
    #   elems [0:512]    = h0   bank 0
    #   elems [512:1024] = h1   bank 1
    # concurrent row-tiled head pair lands in distinct banks.
    def score_mms(pr, sb, region, ck):
        """Scores for ck chunk of (pr, sb) into `region`."""
        nc.tensor.matmul(
            region[:, 0:512],
            lhsT=(kT[pr][0:64, ck * P:(ck + 1) * P]),
            rhs=(qT[pr][0:64, sb * 512:(sb + 1) * 512]),
            start=True, stop=True, tile_position=(0, 0))
        nc.tensor.matmul(
            region[:, 512:1024],
            lhsT=(kT[pr][64:128, ck * P:(ck + 1) * P]),
            rhs=(qT[pr][64:128, sb * 512:(sb + 1) * 512]),
            start=True, stop=True, tile_position=(64, 0))

    def av_mms(pr, ck, etile, accA, accB):
        """AV for ck chunk from exp tile `etile`."""
        nc.tensor.matmul(
            accA, lhsT=(vaug[pr][ck][:, 0, :]),
            rhs=(etile[:, 0:512]),
            start=(ck == 0), stop=(ck == NST - 1))
        nc.tensor.matmul(
            accB, lhsT=(vaug[pr][ck][:, 1, :]),
            rhs=(etile[:, 512:1024]),
            start=(ck == 0), stop=(ck == NST - 1))

    def emit_norm(pr, sb, hh, stg):
        """attnT rows for head hh of (pr, sb) from the staged acc copy."""
        rec = smalls.tile([64, 512], F32, name="rec", tag="rec")
        nc.vector.reciprocal(out=rec, in_=stg[64:128, :])
        nc.vector.tensor_tensor(
            out=attnT[pr][64 * hh:64 * (hh + 1), sb * 512:(sb + 1) * 512],
            in0=stg[0:64, :], in1=rec, op=MULT)

    norm_thunks = []
    proj_thunks = []
    for pr in range(NPAIR):
        side = interleave_gen(pr) if pr < 2 else []
        side_i = 0
        rg_count = 0
        for sb in range(NSQ):
            accA = accP.tile([P, 512], F32, name="accA", tag="accA")
            accB = accP.tile([P, 512], F32, name="accB", tag="accB")
            prev = None
            for j in range(NST):
                region = scoreP.tile([P, 1024], F32, name="score", tag="score")
                score_mms(pr, sb, region, j)
                # interleave qk/v for next pair: pair 0's own v fires every
                # region of sb 0 (AV(ck) needs vaug[0][ck] by region ck+1);
                # the rest every other region.
                rg_count += 1
                if side_i < len(side):
                    if pr == 0 and side_i < NST:
                        side[side_i]()
                        side_i += 1
                    elif rg_count % 2 == 0:
                        side[side_i]()
                        side_i += 1
                # deferred normalize (regions 2-5) and proj (regions 6+) of
                # the PREVIOUS sq block: keeps the slow reciprocal chain and
                # the attnT-dependent proj MMs away from the boundary, where
                # they stall the score/ACT pipeline.
                if 2 <= j <= 5:
                    if norm_thunks:
                        norm_thunks.pop(0)()
                elif j >= 6:
                    for _ in range(min(2, len(proj_thunks))):
                        proj_thunks.pop(0)()
                if prev is not None:
                    av_mms(pr, prev[0], prev[1], accA, accB)
                etile = expp.tile([P, 1024], BF16, name="etile", tag="etile")
                nc.scalar.activation(out=etile, in_=region, func=EXP)
                prev = (j, etile)
            av_mms(pr, prev[0], prev[1], accA, accB)

            if pr == NPAIR - 1 and sb == NSQ - 1:
                # Final block: nothing follows, so normalize straight from
                # PSUM — the staging hop and deferral would only add latency
                # to the tail (norm -> proj -> out DMA).
                for hh, acc in ((0, accA), (1, accB)):
                    emit_norm(pr, sb, hh, acc)
            else:
                # Stage accs to SBUF fast (frees the PSUM banks for the next
                # sq block); reciprocal+mult run deferred, off the critical
                # path.
                for hh, acc in ((0, accA), (1, accB)):
                    stg = smalls.tile([P, 512], F32, name="stg", tag=f"stg{hh}")
                    nc.vector.tensor_copy(out=stg, in_=acc)
                    norm_thunks.append(
                        lambda pr=pr, sb=sb, hh=hh, stg=stg: emit_norm(
                            pr, sb, hh, stg))
            if pr == 2 and sb < NSQ - 1:
                proj_thunks.extend(proj_block(sb))
        # drain leftover side work before next pair needs it
        while side_i < len(side):
            side[side_i]()
            side_i += 1
    # final drain: last normalizes, then the last sq block's proj
    while norm_thunks:
        norm_thunks.pop(0)()
    for t in proj_block(NSQ - 1):
        t()
    while proj_thunks:
        proj_thunks.pop(0)()


_NC_CACHE = None


def _get_nc():
    global _NC_CACHE
    if _NC_CACHE is None:
        _NC_CACHE = build_kernel()
    return _NC_CACHE


def make_in_maps(hidden_states, W_attn, b_attn, W_proj, b_proj):
    in_maps = []
    for c in range(8):
        b, g = c // 2, c % 2
        cols = slice(g * DKC, (g + 1) * DKC)
        wq = W_attn[:, 0 * D:1 * D][:, cols]
        wk = W_attn[:, 1 * D:2 * D][:, cols]
        wv = W_attn[:, 2 * D:3 * D][:, cols]
        bq = b_attn[0 * D:1 * D][cols]
        bk = b_attn[1 * D:2 * D][cols]
        bv = b_attn[2 * D:3 * D][cols]
        in_maps.append({
            "xT": np.ascontiguousarray(hidden_states[b].T).astype(ml_dtypes.bfloat16),
            "wqkv": np.ascontiguousarray(
                np.concatenate([wq, wk, wv], axis=1)).astype(ml_dtypes.bfloat16),
            "bqkv": np.ascontiguousarray(
                np.concatenate([bq, bk, bv]), dtype=np.float32),
            "wproj": np.ascontiguousarray(
                W_proj[g * DKC:(g + 1) * DKC, :]).astype(ml_dtypes.bfloat16),
        })
    return in_maps


def run(hidden_states, W_attn, b_attn, W_proj, b_proj, trace=False):
    nc = _get_nc()
    in_maps = make_in_maps(hidden_states, W_attn, b_attn, W_proj, b_proj)
    res = run_bass_kernel_spmd(nc, in_maps, core_ids=list(range(8)), trace=trace)
    out = np.empty((B, S, D), dtype=np.float32)
    for b in range(B):
        out[b] = res.results[2 * b]["out"] + res.results[2 * b + 1]["out"] + b_proj
    return out, res


def kernel(hidden_states, W_attn, b_attn, W_proj, b_proj):
    hidden_states = np.asarray(hidden_states, dtype=np.float32)
    W_attn = np.asarray(W_attn, dtype=np.float32)
    b_attn = np.asarray(b_attn, dtype=np.float32)
    W_proj = np.asarray(W_proj, dtype=np.float32)
    b_proj = np.asarray(b_proj, dtype=np.float32)
    out, _ = run(hidden_states, W_attn, b_attn, W_proj, b_proj, trace=False)
    return out
